# revision 21
# baseline (speedup 1.0000x reference)
"""Per-row cosine similarity kernel for Trainium2 (Bass/Tile), 8-core SPMD.

Problem: a, b: [64, 2048, 512] fp32 -> out [64, 2048] fp32
  out[i,t] = dot(a,b) / (|a| * |b|)

Sharding: 131072 rows split into 8 contiguous blocks of 16384 rows, one per
NeuronCore (data parallel, no communication).

Architecture (v5, fp16 + PE-reduction + block-contiguous DRAM):
  Inputs staged d-major fp16, block-contiguous: per core a DRAM tensor
  [32, 128, 2048] where element [k, p, h*512+j] = X[k*512 + j, h*128 + p]
  for block k (32 blocks of 512 rows), d-group h (d = h*128 + p). Each
  block's 512 KiB is fully contiguous in HBM, so input DMA runs at large-
  transfer efficiency instead of 4 KiB-strided-descriptor rate.

  The d-reduction runs on the TensorEngine: a ones-selector stationary
  [128, 32] (ones in column k) makes matmul(sel, stat_chunk) accumulate
  block k's per-row sums into PSUM partition k; 4 h-group matmuls per
  stat complete the d=512 reduction, 32 blocks fill a [32, 512] PSUM
  stat tile. (Column-group tile_position packing was tried and produces
  ~1e-3-level corruption in non-zero column groups on this hardware, and
  K=128 matmuls stream strictly serially anyway - so a single group is
  both correct and just as fast.)

  Rejected by measurement: int8 staging (DVE/GPSIMD tensor_tensor on
  8-bit runs ~5x slower than fp16 2x mode, wiping out the DMA savings),
  dual-ring DMA (aggregate HBM rate is capped regardless), col-tiling.

  Engines per core (measured):
    DMA : 32 MiB fp16 in, 64 x 512 KiB contiguous block loads, SP ring
    DVE : prod = a*b, bsq = b*b (fp16 2x, ~1.2 us/block each)
    ACT : asq = a*a (Square 1x, ~1.9 us/block)
    PE  : 3 stats x 32 blocks x 4 h = 384 matmuls N=512 (~215 ns each,
          streaming-bound); ~40 tiny warmup matmuls release the HAM
          clock gate during the DMA ramp
    combine tail: cos = dot * approx_recip(sqrt(na*nb)) on [32, 512]
"""

import os
import sys

import numpy as np

sys.path.insert(0, "/opt/trn_rl_repo")

import concourse.bacc as bacc
import concourse.bass as bass
import concourse.mybir as mybir
import concourse.tile as tile

N_CORES = 8
B, T, D = 64, 2048, 512
ROWS_TOTAL = B * T              # 131072
ROWS_PER_CORE = ROWS_TOTAL // N_CORES  # 16384
P = 128                         # SBUF partitions
NBLK = 32                       # row blocks per core
BLK = ROWS_PER_CORE // NBLK     # 512 rows per block
H = D // P                      # 4 d-groups per row
CW = H * BLK                    # 2048 staged columns per block

F16 = mybir.dt.float16
F32 = mybir.dt.float32
F8 = mybir.dt.float8e4

HALF = NBLK // 2               # stats split into two bank sets; combine
                               # set A mid-kernel to shorten the tail
USE_DR_ASQ = True              # a*a in fp8 + DoubleRow matmuls (half the
                               # PE streaming for that stat)


def _build():
    nc = bacc.Bacc(
        "TRN2",
        target_bir_lowering=False,
        debug=False,
        enable_asserts=False,
        num_devices=N_CORES,
    )
    ab = nc.dram_tensor("ab", [NBLK, P, 2 * CW], F16, kind="ExternalInput").ap()
    o = nc.dram_tensor("o", [ROWS_PER_CORE], F32, kind="ExternalOutput").ap()

    o_v = o.rearrange("(k j) -> k j", k=NBLK)

    with tile.TileContext(nc) as tc:
        with (
            tc.tile_pool(name="io", bufs=6) as io_pool,
            tc.tile_pool(name="pr", bufs=3) as pr_pool,
            tc.tile_pool(name="sq", bufs=2) as sq_pool,
            tc.tile_pool(name="ps", bufs=1, space=bass.MemorySpace.PSUM) as ps_pool,
            tc.tile_pool(name="fin", bufs=1) as fin_pool,
        ):
            # ones-selector: sel[p, c] = 1 iff c == 31; block k (= set
            # column m = k%16) takes the window sel[:, 31-m : 47-m] so the
            # ones land in column m. sel8 is the fp8 DoubleRow twin with
            # the selector duplicated across the Ko=2 interleave dim.
            sel = fin_pool.tile([P, 64], F16, tag="sel")
            nc.vector.memset(sel[:], 0.0)
            nc.vector.memset(sel[:, 31:32], 1.0)
            if USE_DR_ASQ:
                sel8 = fin_pool.tile([P, 2, 64], F8, tag="sel8")
                nc.vector.memset(sel8[:], 0.0)
                nc.vector.memset(sel8[:, :, 31:32], 1.0)

            # warm the sqrt table set during the DMA ramp; Square is a
            # filler fn in the same set -> no reload later.
            warm = fin_pool.tile([P, 1], F32, tag="warm")
            nc.vector.memset(warm[:], 1.0)
            nc.scalar.sqrt(warm[:], warm[:])

            # stats in two bank sets: blocks 0..15 -> set 0 (combined and
            # written out mid-kernel), 16..31 -> set 1 (tail). Full-bank
            # [128, 512] allocations keep PE writes and combine reads in
            # disjoint PSUM banks; only partitions 0..15 are used.
            dot_ps = [
                ps_pool.tile([P, BLK], F32, tag=f"dot{s}", name=f"dot{s}")
                for s in range(2)
            ]
            na_ps = [
                ps_pool.tile([P, BLK], F32, tag=f"na{s}", name=f"na{s}")
                for s in range(2)
            ]
            nb_ps = [
                ps_pool.tile([P, BLK], F32, tag=f"nb{s}", name=f"nb{s}")
                for s in range(2)
            ]
            junk_ps = ps_pool.tile([NBLK, 64], F32, tag="junk")

            # HAM warmup: ~40 tiny matmuls keep PE busy ~3.4us during the
            # DMA ramp so real matmuls start at 2.4 GHz (K=8/8).
            for _ in range(40):
                nc.tensor.matmul(
                    junk_ps[:, 0:32], sel[:, 0:32], sel[:, 0:32],
                    start=True, stop=True,
                )

            def combine(s):
                """cos = dot * recip(sqrt(na*nb)) for bank set s -> DRAM.

                TensorTensor reads at most one PSUM operand, so na goes
                through SBUF via ACT (which sits close to PSUM)."""
                na_sb = fin_pool.tile([HALF, BLK], F32, tag=f"na_sb{s}")
                nc.scalar.copy(na_sb[:], na_ps[s][0:HALF, :])
                pr = fin_pool.tile([HALF, BLK], F32, tag=f"pr{s}")
                nc.vector.tensor_mul(pr[:], na_sb[:], nb_ps[s][0:HALF, :])
                rt = fin_pool.tile([HALF, BLK], F32, tag=f"rt{s}")
                nc.scalar.sqrt(rt[:], pr[:])
                inv = fin_pool.tile([HALF, BLK], F32, tag=f"inv{s}")
                nc.vector.reciprocal_approx_fast(inv[:], rt[:])
                res = fin_pool.tile([HALF, BLK], F32, tag=f"res{s}")
                nc.vector.tensor_mul(res[:], dot_ps[s][0:HALF, :], inv[:])
                nc.sync.dma_start(o_v[s * HALF:(s + 1) * HALF, :], res[:])

            for k in range(NBLK):
                # one fully contiguous 1 MiB load per block: per partition
                # 8 KiB [a_p | b_p], single DMA on the SP ring (a second
                # ring or stream halves per-stream HBM sequential rate)
                ab_t = io_pool.tile([P, 2 * CW], F16, tag="ab")
                nc.sync.dma_start(ab_t[:], ab[k])
                a_t = ab_t[:, 0:CW]
                b_t = ab_t[:, CW:2 * CW]

                prod = pr_pool.tile([P, CW], F16, tag="prod")
                nc.vector.tensor_mul(prod[:], a_t, b_t)
                bsq = pr_pool.tile([P, CW], F16, tag="bsq")
                nc.vector.tensor_mul(bsq[:], b_t, b_t)
                # pre-reduce the dot stream on the otherwise-idle GPSIMD:
                # pair-sum halves the PE streaming for the dot stat
                # (any d-pairing works - it all sums to the same total)
                ph = pr_pool.tile([P, CW // 2], F16, tag="ph")
                nc.gpsimd.tensor_add(ph[:], prod[:, 0:CW // 2], prod[:, CW // 2:CW])

                s = k // HALF
                m = k % HALF
                w = sel[:, 31 - m:47 - m]
                st = m == 0
                sp = m == HALF - 1
                out16 = slice(0, HALF)
                if USE_DR_ASQ:
                    # a*a in fp8: half the PE streaming via DoubleRow
                    # (2 fp8 weights/cell -> contraction 256 = 2 h-groups)
                    asq = sq_pool.tile([P, CW], F8, tag="asq")
                    nc.scalar.activation(
                        asq[:], a_t, mybir.ActivationFunctionType.Square
                    )
                    w8 = sel8[:, :, 31 - m:47 - m]
                    for g in range(2):
                        rv = asq[:, g * 2 * BLK:(g + 1) * 2 * BLK].rearrange(
                            "p (ko f) -> p ko f", ko=2
                        )
                        nc.tensor.matmul(
                            na_ps[s][out16, :], w8, rv,
                            start=st and g == 0, stop=sp and g == 1,
                            perf_mode=mybir.MatmulPerfMode.DoubleRow,
                        )
                else:
                    asq = sq_pool.tile([P, CW], F16, tag="asq")
                    nc.scalar.activation(
                        asq[:], a_t, mybir.ActivationFunctionType.Square
                    )
                    for h in range(H):
                        hs = slice(h * BLK, (h + 1) * BLK)
                        nc.tensor.matmul(
                            na_ps[s][out16, :], w, asq[:, hs],
                            start=st and h == 0, stop=sp and h == H - 1,
                        )

                for g in range(2):
                    gs = slice(g * BLK, (g + 1) * BLK)
                    nc.tensor.matmul(
                        dot_ps[s][out16, :], w, ph[:, gs],
                        start=st and g == 0, stop=sp and g == 1,
                    )
                for h in range(H):
                    hs = slice(h * BLK, (h + 1) * BLK)
                    nc.tensor.matmul(
                        nb_ps[s][out16, :], w, bsq[:, hs],
                        start=st and h == 0, stop=sp and h == H - 1,
                    )

                if k == HALF - 1:
                    combine(0)

            combine(1)

    nc.compile()
    return nc


_NC = None


def _get_nc():
    global _NC
    if _NC is None:
        _NC = _build()
    return _NC


def _run_prestaged(nc, *full_inputs: np.ndarray) -> np.ndarray:
    """Execute the SPMD program on 8 cores with inputs pre-staged as sharded
    device arrays. Staging first (and blocking on it) keeps host->HBM input
    DMA out of the execution window."""
    import jax
    from jax.sharding import Mesh, NamedSharding, PartitionSpec
    from jax.experimental.shard_map import shard_map

    from concourse.bass2jax import (
        _bass_exec_p,
        install_neuronx_cc_hook,
        partition_id_tensor,
    )

    install_neuronx_cc_hook()
    assert nc.dbg_addr is None

    partition_name = (
        nc.partition_id_tensor.name if nc.partition_id_tensor else None
    )
    in_names = []
    out_names = []
    out_avals = []
    zero_outs = []
    for alloc in nc.m.functions[0].allocations:
        if not isinstance(alloc, mybir.MemoryLocationSet):
            continue
        name = alloc.memorylocations[0].name
        if alloc.kind == "ExternalInput":
            if name != partition_name:
                in_names.append(name)
        elif alloc.kind == "ExternalOutput":
            out_names.append(name)
            shape = tuple(alloc.tensor_shape)
            dtype = mybir.dt.np(alloc.dtype)
            out_avals.append(jax.core.ShapedArray(shape, dtype))
            zero_outs.append(np.zeros((N_CORES * shape[0], *shape[1:]), dtype))
    n_params = len(in_names)
    all_names = list(in_names + out_names)
    if partition_name is not None:
        all_names.append(partition_name)
    donate = tuple(range(n_params, n_params + len(out_names)))

    def _body(*args):
        operands = list(args)
        if partition_name is not None:
            operands.append(partition_id_tensor())
        return tuple(
            _bass_exec_p.bind(
                *operands,
                out_avals=tuple(out_avals),
                in_names=tuple(all_names),
                out_names=tuple(out_names),
                lowering_input_output_aliases=(),
                sim_require_finite=True,
                sim_require_nnan=True,
                nc=nc,
            )
        )

    devices = jax.devices()[:N_CORES]
    mesh = Mesh(np.asarray(devices), ("core",))
    spec = NamedSharding(mesh, PartitionSpec("core"))
    n_in = n_params + len(out_names)
    sharded = jax.jit(
        shard_map(
            _body,
            mesh=mesh,
            in_specs=(PartitionSpec("core"),) * n_in,
            out_specs=(PartitionSpec("core"),) * len(out_names),
            check_rep=False,
        ),
        donate_argnums=donate,
        keep_unused=True,
    )
    # in_names order matches dram_tensor declaration order
    staged = [
        jax.device_put(arr, spec)
        for arr in (*full_inputs, *zero_outs)
    ]
    jax.block_until_ready(staged)
    out_arrs = sharded(*staged)
    return np.asarray(out_arrs[0])


def _stage_ab(a: np.ndarray, b: np.ndarray) -> np.ndarray:
    """2x [131072, 512] fp32 -> [256, 128, 4096] fp16 interleaved staging.

    d-major per block: v[core*32 + k, p, h*512 + j] = X[(core*32+k)*512
    + j, h*128 + p] for X in {a (cols 0:2048), b (cols 2048:4096)}. Each
    block is one fully contiguous 1 MiB DRAM region ([a_p | b_p] 8 KiB
    per partition), so input DMA is a single large sequential read per
    block; the d-axis reduction maps to SBUF partitions so the
    TensorEngine can do it with a ones-selector stationary."""
    def dmaj(x):
        v = x.astype(np.float16).reshape(N_CORES * NBLK, BLK, H, P)
        return v.transpose(0, 3, 2, 1).reshape(N_CORES * NBLK, P, CW)
    return np.ascontiguousarray(
        np.concatenate([dmaj(a), dmaj(b)], axis=2)
    )


def kernel(a: np.ndarray, b: np.ndarray) -> np.ndarray:
    nc = _get_nc()
    abf = _stage_ab(
        np.asarray(a, dtype=np.float32).reshape(ROWS_TOTAL, D),
        np.asarray(b, dtype=np.float32).reshape(ROWS_TOTAL, D),
    )
    out = _run_prestaged(nc, abf)
    return out.reshape(B, T).astype(np.float32)


# revision 23
# speedup vs baseline: 1.4973x; 1.4973x over previous
"""Per-row cosine similarity kernel for Trainium2 (Bass/Tile), 8-core SPMD.

Problem: a, b: [64, 2048, 512] fp32 -> out [64, 2048] fp32
  out[i,t] = dot(a,b) / (|a| * |b|)

Sharding: 131072 rows split into 8 contiguous blocks of 16384 rows, one per
NeuronCore (data parallel, no communication).

Architecture (v7: fp16 interleaved staging + PE ones-reduction):
  Inputs staged d-major fp16 and interleaved: per core one DRAM tensor
  [32, 128, 4096] where block k holds [a_p | b_p] per partition, with
  [k, p, h*512+j] = X[k*512 + j, h*128 + p] (blocks of 512 rows, d-group
  h, d = h*128 + p). Each block is one fully contiguous 1 MiB region, so
  input DMA is a single large sequential read per block on one queue
  (~400 GB/s measured vs ~330 for 4 KiB-strided descriptors; a second
  parallel stream HALVES per-stream rate - never split).

  The d-reduction runs on the TensorEngine: a ones-selector stationary
  (ones in column m) makes matmul(sel, stat_chunk) accumulate block k's
  per-row sums into PSUM partition m = k%16; h-group matmuls PSUM-
  accumulate the full d=512 reduction. Stats live in two bank sets
  (blocks 0-15 / 16-31) so the first half combines and streams out
  mid-kernel, shortening the tail. The a*a stat is computed in fp8 and
  reduced with DoubleRow matmuls (2 fp8 weights/cell -> contraction 256,
  half the PE streaming; the ones-reduction is pairing-convention-proof).

  Rejected by measurement: int8/8-bit staging (DVE/GPSIMD tensor_tensor
  on 8-bit inputs ~5x slower than fp16 2x), dual-ring DMA (each stream
  drops to ~206 B/ns), tile_position col-tiling (K=128 matmuls stream
  serially anyway AND non-zero column groups corrupt results ~1e-3),
  GPSIMD fp16 adds (~6 us/block).

  Engines per core (measured, per 512-row block):
    DMA : 1 MiB contiguous load, ~2.6 us (fast cores; per-core HBM rate
          varies 326-402 B/ns run to run)
    DVE : prod = a*b, bsq = b*b (fp16 2x, ~1.2 us each)
    ACT : asq = a*a -> fp8 (Square 1x, ~1.8 us)
    PE  : dot 4 + nb 4 (fp16 N=512) + na 2 (fp8 DoubleRow) matmuls,
          ~3.0 us; 40 tiny warmup matmuls release the HAM clock gate
          during the DMA ramp
    combine: cos = dot * approx_recip(sqrt(na*nb)), [16, 512] x 2 sets
"""

import os
import sys

import numpy as np

sys.path.insert(0, "/opt/trn_rl_repo")

import concourse.bacc as bacc
import concourse.bass as bass
import concourse.mybir as mybir
import concourse.tile as tile

N_CORES = 8
B, T, D = 64, 2048, 512
ROWS_TOTAL = B * T              # 131072
ROWS_PER_CORE = ROWS_TOTAL // N_CORES  # 16384
P = 128                         # SBUF partitions
NBLK = 32                       # row blocks per core
BLK = ROWS_PER_CORE // NBLK     # 512 rows per block
H = D // P                      # 4 d-groups per row
CW = H * BLK                    # 2048 staged columns per block

F16 = mybir.dt.float16
F32 = mybir.dt.float32
F8 = mybir.dt.float8e4

HALF = NBLK // 2               # stats split into two bank sets; combine
                               # set A mid-kernel to shorten the tail
USE_DR_ASQ = True              # a*a in fp8 + DoubleRow matmuls (half the
                               # PE streaming for that stat)


def _build():
    nc = bacc.Bacc(
        "TRN2",
        target_bir_lowering=False,
        debug=False,
        enable_asserts=False,
        num_devices=N_CORES,
    )
    ab = nc.dram_tensor("ab", [NBLK, P, 2 * CW], F16, kind="ExternalInput").ap()
    o = nc.dram_tensor("o", [ROWS_PER_CORE], F32, kind="ExternalOutput").ap()

    o_v = o.rearrange("(k j) -> k j", k=NBLK)

    with tile.TileContext(nc) as tc:
        with (
            tc.tile_pool(name="io", bufs=5) as io_pool,
            tc.tile_pool(name="pr", bufs=3) as pr_pool,
            tc.tile_pool(name="sq", bufs=2) as sq_pool,
            tc.tile_pool(name="ps", bufs=1, space=bass.MemorySpace.PSUM) as ps_pool,
            tc.tile_pool(name="fin", bufs=1) as fin_pool,
        ):
            # ones-selector: sel[p, c] = 1 iff c == 31; block k (= set
            # column m = k%16) takes the window sel[:, 31-m : 47-m] so the
            # ones land in column m. sel8 is the fp8 DoubleRow twin with
            # the selector duplicated across the Ko=2 interleave dim.
            sel = fin_pool.tile([P, 64], F16, tag="sel")
            nc.vector.memset(sel[:], 0.0)
            nc.vector.memset(sel[:, 31:32], 1.0)
            if USE_DR_ASQ:
                sel8 = fin_pool.tile([P, 2, 64], F8, tag="sel8")
                nc.vector.memset(sel8[:], 0.0)
                nc.vector.memset(sel8[:, :, 31:32], 1.0)

            # warm the sqrt table set during the DMA ramp; Square is a
            # filler fn in the same set -> no reload later.
            warm = fin_pool.tile([P, 1], F32, tag="warm")
            nc.vector.memset(warm[:], 1.0)
            nc.scalar.sqrt(warm[:], warm[:])

            # stats in two bank sets: blocks 0..15 -> set 0 (combined and
            # written out mid-kernel), 16..31 -> set 1 (tail). Full-bank
            # [128, 512] allocations keep PE writes and combine reads in
            # disjoint PSUM banks; only partitions 0..15 are used.
            dot_ps = [
                ps_pool.tile([P, BLK], F32, tag=f"dot{s}", name=f"dot{s}")
                for s in range(2)
            ]
            na_ps = [
                ps_pool.tile([P, BLK], F32, tag=f"na{s}", name=f"na{s}")
                for s in range(2)
            ]
            nb_ps = [
                ps_pool.tile([P, BLK], F32, tag=f"nb{s}", name=f"nb{s}")
                for s in range(2)
            ]
            junk_ps = ps_pool.tile([NBLK, 64], F32, tag="junk")

            # HAM warmup: ~40 tiny matmuls keep PE busy ~3.4us during the
            # DMA ramp so real matmuls start at 2.4 GHz (K=8/8).
            for _ in range(40):
                nc.tensor.matmul(
                    junk_ps[:, 0:32], sel[:, 0:32], sel[:, 0:32],
                    start=True, stop=True,
                )

            def combine(s):
                """cos = dot * recip(sqrt(na*nb)) for bank set s -> DRAM.

                TensorTensor reads at most one PSUM operand, so na goes
                through SBUF via ACT (which sits close to PSUM)."""
                na_sb = fin_pool.tile([HALF, BLK], F32, tag=f"na_sb{s}")
                nc.scalar.copy(na_sb[:], na_ps[s][0:HALF, :])
                pr = fin_pool.tile([HALF, BLK], F32, tag=f"pr{s}")
                nc.vector.tensor_mul(pr[:], na_sb[:], nb_ps[s][0:HALF, :])
                rt = fin_pool.tile([HALF, BLK], F32, tag=f"rt{s}")
                nc.scalar.sqrt(rt[:], pr[:])
                inv = fin_pool.tile([HALF, BLK], F32, tag=f"inv{s}")
                nc.vector.reciprocal_approx_fast(inv[:], rt[:])
                res = fin_pool.tile([HALF, BLK], F32, tag=f"res{s}")
                nc.vector.tensor_mul(res[:], dot_ps[s][0:HALF, :], inv[:])
                nc.sync.dma_start(o_v[s * HALF:(s + 1) * HALF, :], res[:])

            for k in range(NBLK):
                # one fully contiguous 1 MiB load per block: per partition
                # 8 KiB [a_p | b_p], single DMA on the SP ring (a second
                # ring or stream halves per-stream HBM sequential rate)
                ab_t = io_pool.tile([P, 2 * CW], F16, tag="ab")
                nc.sync.dma_start(ab_t[:], ab[k])
                a_t = ab_t[:, 0:CW]
                b_t = ab_t[:, CW:2 * CW]

                prod = pr_pool.tile([P, CW], F16, tag="prod")
                nc.vector.tensor_mul(prod[:], a_t, b_t)
                bsq = pr_pool.tile([P, CW], F16, tag="bsq")
                nc.vector.tensor_mul(bsq[:], b_t, b_t)

                s = k // HALF
                m = k % HALF
                w = sel[:, 31 - m:47 - m]
                st = m == 0
                sp = m == HALF - 1
                out16 = slice(0, HALF)
                if USE_DR_ASQ:
                    # a*a in fp8: half the PE streaming via DoubleRow
                    # (2 fp8 weights/cell -> contraction 256 = 2 h-groups)
                    asq = sq_pool.tile([P, CW], F8, tag="asq")
                    nc.scalar.activation(
                        asq[:], a_t, mybir.ActivationFunctionType.Square
                    )
                    w8 = sel8[:, :, 31 - m:47 - m]
                    for g in range(2):
                        rv = asq[:, g * 2 * BLK:(g + 1) * 2 * BLK].rearrange(
                            "p (ko f) -> p ko f", ko=2
                        )
                        nc.tensor.matmul(
                            na_ps[s][out16, :], w8, rv,
                            start=st and g == 0, stop=sp and g == 1,
                            perf_mode=mybir.MatmulPerfMode.DoubleRow,
                        )
                else:
                    asq = sq_pool.tile([P, CW], F16, tag="asq")
                    nc.scalar.activation(
                        asq[:], a_t, mybir.ActivationFunctionType.Square
                    )
                    for h in range(H):
                        hs = slice(h * BLK, (h + 1) * BLK)
                        nc.tensor.matmul(
                            na_ps[s][out16, :], w, asq[:, hs],
                            start=st and h == 0, stop=sp and h == H - 1,
                        )

                for h in range(H):
                    hs = slice(h * BLK, (h + 1) * BLK)
                    nc.tensor.matmul(
                        dot_ps[s][out16, :], w, prod[:, hs],
                        start=st and h == 0, stop=sp and h == H - 1,
                    )
                    nc.tensor.matmul(
                        nb_ps[s][out16, :], w, bsq[:, hs],
                        start=st and h == 0, stop=sp and h == H - 1,
                    )

                if k == HALF - 1:
                    combine(0)

            combine(1)

    nc.compile()
    return nc


_NC = None


def _get_nc():
    global _NC
    if _NC is None:
        _NC = _build()
    return _NC


def _run_prestaged(nc, *full_inputs: np.ndarray) -> np.ndarray:
    """Execute the SPMD program on 8 cores with inputs pre-staged as sharded
    device arrays. Staging first (and blocking on it) keeps host->HBM input
    DMA out of the execution window."""
    import jax
    from jax.sharding import Mesh, NamedSharding, PartitionSpec
    from jax.experimental.shard_map import shard_map

    from concourse.bass2jax import (
        _bass_exec_p,
        install_neuronx_cc_hook,
        partition_id_tensor,
    )

    install_neuronx_cc_hook()
    assert nc.dbg_addr is None

    partition_name = (
        nc.partition_id_tensor.name if nc.partition_id_tensor else None
    )
    in_names = []
    out_names = []
    out_avals = []
    zero_outs = []
    for alloc in nc.m.functions[0].allocations:
        if not isinstance(alloc, mybir.MemoryLocationSet):
            continue
        name = alloc.memorylocations[0].name
        if alloc.kind == "ExternalInput":
            if name != partition_name:
                in_names.append(name)
        elif alloc.kind == "ExternalOutput":
            out_names.append(name)
            shape = tuple(alloc.tensor_shape)
            dtype = mybir.dt.np(alloc.dtype)
            out_avals.append(jax.core.ShapedArray(shape, dtype))
            zero_outs.append(np.zeros((N_CORES * shape[0], *shape[1:]), dtype))
    n_params = len(in_names)
    all_names = list(in_names + out_names)
    if partition_name is not None:
        all_names.append(partition_name)
    donate = tuple(range(n_params, n_params + len(out_names)))

    def _body(*args):
        operands = list(args)
        if partition_name is not None:
            operands.append(partition_id_tensor())
        return tuple(
            _bass_exec_p.bind(
                *operands,
                out_avals=tuple(out_avals),
                in_names=tuple(all_names),
                out_names=tuple(out_names),
                lowering_input_output_aliases=(),
                sim_require_finite=True,
                sim_require_nnan=True,
                nc=nc,
            )
        )

    devices = jax.devices()[:N_CORES]
    mesh = Mesh(np.asarray(devices), ("core",))
    spec = NamedSharding(mesh, PartitionSpec("core"))
    n_in = n_params + len(out_names)
    sharded = jax.jit(
        shard_map(
            _body,
            mesh=mesh,
            in_specs=(PartitionSpec("core"),) * n_in,
            out_specs=(PartitionSpec("core"),) * len(out_names),
            check_rep=False,
        ),
        donate_argnums=donate,
        keep_unused=True,
    )
    # in_names order matches dram_tensor declaration order
    staged = [
        jax.device_put(arr, spec)
        for arr in (*full_inputs, *zero_outs)
    ]
    jax.block_until_ready(staged)
    out_arrs = sharded(*staged)
    return np.asarray(out_arrs[0])


def _stage_ab(a: np.ndarray, b: np.ndarray) -> np.ndarray:
    """2x [131072, 512] fp32 -> [256, 128, 4096] fp16 interleaved staging.

    d-major per block: v[core*32 + k, p, h*512 + j] = X[(core*32+k)*512
    + j, h*128 + p] for X in {a (cols 0:2048), b (cols 2048:4096)}. Each
    block is one fully contiguous 1 MiB DRAM region ([a_p | b_p] 8 KiB
    per partition), so input DMA is a single large sequential read per
    block; the d-axis reduction maps to SBUF partitions so the
    TensorEngine can do it with a ones-selector stationary."""
    def dmaj(x):
        v = x.astype(np.float16).reshape(N_CORES * NBLK, BLK, H, P)
        return v.transpose(0, 3, 2, 1).reshape(N_CORES * NBLK, P, CW)
    return np.ascontiguousarray(
        np.concatenate([dmaj(a), dmaj(b)], axis=2)
    )


def kernel(a: np.ndarray, b: np.ndarray) -> np.ndarray:
    nc = _get_nc()
    abf = _stage_ab(
        np.asarray(a, dtype=np.float32).reshape(ROWS_TOTAL, D),
        np.asarray(b, dtype=np.float32).reshape(ROWS_TOTAL, D),
    )
    out = _run_prestaged(nc, abf)
    return out.reshape(B, T).astype(np.float32)


# revision 24
# speedup vs baseline: 1.7238x; 1.1513x over previous
"""Per-row cosine similarity kernel for Trainium2 (Bass/Tile), 8-core SPMD.

Problem: a, b: [64, 2048, 512] fp32 -> out [64, 2048] fp32
  out[i,t] = dot(a,b) / (|a| * |b|)

Sharding: 131072 rows split into 8 contiguous blocks of 16384 rows, one per
NeuronCore (data parallel, no communication).

Architecture (v7: fp16 interleaved staging + PE ones-reduction):
  Inputs staged d-major fp16 and interleaved: per core one DRAM tensor
  [32, 128, 4096] where block k holds [a_p | b_p] per partition, with
  [k, p, h*512+j] = X[k*512 + j, h*128 + p] (blocks of 512 rows, d-group
  h, d = h*128 + p). Each block is one fully contiguous 1 MiB region, so
  input DMA is a single large sequential read per block on one queue
  (~400 GB/s measured vs ~330 for 4 KiB-strided descriptors; a second
  parallel stream HALVES per-stream rate - never split).

  The d-reduction runs on the TensorEngine: a ones-selector stationary
  (ones in column m) makes matmul(sel, stat_chunk) accumulate block k's
  per-row sums into PSUM partition m = k%16; h-group matmuls PSUM-
  accumulate the full d=512 reduction. Stats live in two bank sets
  (blocks 0-15 / 16-31) so the first half combines and streams out
  mid-kernel, shortening the tail. The a*a stat is computed in fp8 and
  reduced with DoubleRow matmuls (2 fp8 weights/cell -> contraction 256,
  half the PE streaming; the ones-reduction is pairing-convention-proof).

  Rejected by measurement: int8/8-bit staging (DVE/GPSIMD tensor_tensor
  on 8-bit inputs ~5x slower than fp16 2x), dual-ring DMA (each stream
  drops to ~206 B/ns), tile_position col-tiling (K=128 matmuls stream
  serially anyway AND non-zero column groups corrupt results ~1e-3),
  GPSIMD fp16 adds (~6 us/block).

  Engines per core (measured, per 512-row block):
    DMA : 1 MiB contiguous load, ~2.6 us (fast cores; per-core HBM rate
          varies 326-402 B/ns run to run)
    DVE : prod = a*b, bsq = b*b (fp16 2x, ~1.2 us each)
    ACT : asq = a*a -> fp8 (Square 1x, ~1.8 us)
    PE  : dot 4 + nb 4 (fp16 N=512) + na 2 (fp8 DoubleRow) matmuls,
          ~3.0 us; 40 tiny warmup matmuls release the HAM clock gate
          during the DMA ramp
    combine: cos = dot * approx_recip(sqrt(na*nb)), [16, 512] x 2 sets
"""

import os
import sys

import numpy as np

sys.path.insert(0, "/opt/trn_rl_repo")

import concourse.bacc as bacc
import concourse.bass as bass
import concourse.mybir as mybir
import concourse.tile as tile

N_CORES = 8
B, T, D = 64, 2048, 512
ROWS_TOTAL = B * T              # 131072
ROWS_PER_CORE = ROWS_TOTAL // N_CORES  # 16384
P = 128                         # SBUF partitions
NBLK = 32                       # row blocks per core
BLK = ROWS_PER_CORE // NBLK     # 512 rows per block
H = D // P                      # 4 d-groups per row
CW = H * BLK                    # 2048 staged columns per block

F16 = mybir.dt.float16
F32 = mybir.dt.float32
F8 = mybir.dt.float8e4

HALF = NBLK // 2               # stats split into two bank sets; combine
                               # set A mid-kernel to shorten the tail
USE_DR_ASQ = True              # a*a in fp8 + DoubleRow matmuls (half the
                               # PE streaming for that stat)


def _build():
    nc = bacc.Bacc(
        "TRN2",
        target_bir_lowering=False,
        debug=False,
        enable_asserts=False,
        num_devices=N_CORES,
    )
    ab = nc.dram_tensor("ab", [NBLK, P, 2 * CW], F16, kind="ExternalInput").ap()
    o = nc.dram_tensor("o", [ROWS_PER_CORE], F32, kind="ExternalOutput").ap()

    o_v = o.rearrange("(k j) -> k j", k=NBLK)

    with tile.TileContext(nc) as tc:
        with (
            tc.tile_pool(name="io", bufs=5) as io_pool,
            tc.tile_pool(name="pr", bufs=8) as pr_pool,
            tc.tile_pool(name="sq", bufs=6) as sq_pool,
            tc.tile_pool(name="ps", bufs=1, space=bass.MemorySpace.PSUM) as ps_pool,
            tc.tile_pool(name="fin", bufs=1) as fin_pool,
        ):
            # ones-selector: sel[p, c] = 1 iff c == 31; block k (= set
            # column m = k%16) takes the window sel[:, 31-m : 47-m] so the
            # ones land in column m. sel8 is the fp8 DoubleRow twin with
            # the selector duplicated across the Ko=2 interleave dim.
            sel = fin_pool.tile([P, 64], F16, tag="sel")
            nc.vector.memset(sel[:], 0.0)
            nc.vector.memset(sel[:, 31:32], 1.0)
            if USE_DR_ASQ:
                sel8 = fin_pool.tile([P, 2, 64], F8, tag="sel8")
                nc.vector.memset(sel8[:], 0.0)
                nc.vector.memset(sel8[:, :, 31:32], 1.0)

            # warm the sqrt table set during the DMA ramp; Square is a
            # filler fn in the same set -> no reload later.
            warm = fin_pool.tile([P, 1], F32, tag="warm")
            nc.vector.memset(warm[:], 1.0)
            nc.scalar.sqrt(warm[:], warm[:])

            # stats in two bank sets: blocks 0..15 -> set 0 (combined and
            # written out mid-kernel), 16..31 -> set 1 (tail). Full-bank
            # [128, 512] allocations keep PE writes and combine reads in
            # disjoint PSUM banks; only partitions 0..15 are used.
            dot_ps = [
                ps_pool.tile([P, BLK], F32, tag=f"dot{s}", name=f"dot{s}")
                for s in range(2)
            ]
            na_ps = [
                ps_pool.tile([P, BLK], F32, tag=f"na{s}", name=f"na{s}")
                for s in range(2)
            ]
            nb_ps = [
                ps_pool.tile([P, BLK], F32, tag=f"nb{s}", name=f"nb{s}")
                for s in range(2)
            ]
            junk_ps = ps_pool.tile([NBLK, 64], F32, tag="junk")

            # HAM warmup: ~40 tiny matmuls keep PE busy ~3.4us during the
            # DMA ramp so real matmuls start at 2.4 GHz (K=8/8).
            for _ in range(40):
                nc.tensor.matmul(
                    junk_ps[:, 0:32], sel[:, 0:32], sel[:, 0:32],
                    start=True, stop=True,
                )

            def combine(s):
                """cos = dot * recip(sqrt(na*nb)) for bank set s -> DRAM.

                TensorTensor reads at most one PSUM operand, so na goes
                through SBUF via ACT (which sits close to PSUM)."""
                na_sb = fin_pool.tile([HALF, BLK], F32, tag=f"na_sb{s}")
                nc.scalar.copy(na_sb[:], na_ps[s][0:HALF, :])
                pr = fin_pool.tile([HALF, BLK], F32, tag=f"pr{s}")
                nc.vector.tensor_mul(pr[:], na_sb[:], nb_ps[s][0:HALF, :])
                rt = fin_pool.tile([HALF, BLK], F32, tag=f"rt{s}")
                nc.scalar.sqrt(rt[:], pr[:])
                inv = fin_pool.tile([HALF, BLK], F32, tag=f"inv{s}")
                nc.vector.reciprocal_approx_fast(inv[:], rt[:])
                res = fin_pool.tile([HALF, BLK], F32, tag=f"res{s}")
                nc.vector.tensor_mul(res[:], dot_ps[s][0:HALF, :], inv[:])
                nc.sync.dma_start(o_v[s * HALF:(s + 1) * HALF, :], res[:])

            for k in range(NBLK):
                # one fully contiguous 1 MiB load per block: per partition
                # 8 KiB [a_p | b_p], single DMA on the SP ring (a second
                # ring or stream halves per-stream HBM sequential rate)
                ab_t = io_pool.tile([P, 2 * CW], F16, tag="ab")
                nc.sync.dma_start(ab_t[:], ab[k])
                a_t = ab_t[:, 0:CW]
                b_t = ab_t[:, CW:2 * CW]

                prod = pr_pool.tile([P, CW], F16, tag="prod")
                nc.vector.tensor_mul(prod[:], a_t, b_t)
                bsq = pr_pool.tile([P, CW], F16, tag="bsq")
                nc.vector.tensor_mul(bsq[:], b_t, b_t)

                s = k // HALF
                m = k % HALF
                w = sel[:, 31 - m:47 - m]
                st = m == 0
                sp = m == HALF - 1
                out16 = slice(0, HALF)
                if USE_DR_ASQ:
                    # a*a in fp8: half the PE streaming via DoubleRow
                    # (2 fp8 weights/cell -> contraction 256 = 2 h-groups)
                    asq = sq_pool.tile([P, CW], F8, tag="asq")
                    nc.scalar.activation(
                        asq[:], a_t, mybir.ActivationFunctionType.Square
                    )
                    w8 = sel8[:, :, 31 - m:47 - m]
                    for g in range(2):
                        rv = asq[:, g * 2 * BLK:(g + 1) * 2 * BLK].rearrange(
                            "p (ko f) -> p ko f", ko=2
                        )
                        nc.tensor.matmul(
                            na_ps[s][out16, :], w8, rv,
                            start=st and g == 0, stop=sp and g == 1,
                            perf_mode=mybir.MatmulPerfMode.DoubleRow,
                        )
                else:
                    asq = sq_pool.tile([P, CW], F16, tag="asq")
                    nc.scalar.activation(
                        asq[:], a_t, mybir.ActivationFunctionType.Square
                    )
                    for h in range(H):
                        hs = slice(h * BLK, (h + 1) * BLK)
                        nc.tensor.matmul(
                            na_ps[s][out16, :], w, asq[:, hs],
                            start=st and h == 0, stop=sp and h == H - 1,
                        )

                for h in range(H):
                    hs = slice(h * BLK, (h + 1) * BLK)
                    nc.tensor.matmul(
                        dot_ps[s][out16, :], w, prod[:, hs],
                        start=st and h == 0, stop=sp and h == H - 1,
                    )
                    nc.tensor.matmul(
                        nb_ps[s][out16, :], w, bsq[:, hs],
                        start=st and h == 0, stop=sp and h == H - 1,
                    )

                if k == HALF - 1:
                    combine(0)

            combine(1)

    nc.compile()
    return nc


_NC = None


def _get_nc():
    global _NC
    if _NC is None:
        _NC = _build()
    return _NC


def _run_prestaged(nc, *full_inputs: np.ndarray) -> np.ndarray:
    """Execute the SPMD program on 8 cores with inputs pre-staged as sharded
    device arrays. Staging first (and blocking on it) keeps host->HBM input
    DMA out of the execution window."""
    import jax
    from jax.sharding import Mesh, NamedSharding, PartitionSpec
    from jax.experimental.shard_map import shard_map

    from concourse.bass2jax import (
        _bass_exec_p,
        install_neuronx_cc_hook,
        partition_id_tensor,
    )

    install_neuronx_cc_hook()
    assert nc.dbg_addr is None

    partition_name = (
        nc.partition_id_tensor.name if nc.partition_id_tensor else None
    )
    in_names = []
    out_names = []
    out_avals = []
    zero_outs = []
    for alloc in nc.m.functions[0].allocations:
        if not isinstance(alloc, mybir.MemoryLocationSet):
            continue
        name = alloc.memorylocations[0].name
        if alloc.kind == "ExternalInput":
            if name != partition_name:
                in_names.append(name)
        elif alloc.kind == "ExternalOutput":
            out_names.append(name)
            shape = tuple(alloc.tensor_shape)
            dtype = mybir.dt.np(alloc.dtype)
            out_avals.append(jax.core.ShapedArray(shape, dtype))
            zero_outs.append(np.zeros((N_CORES * shape[0], *shape[1:]), dtype))
    n_params = len(in_names)
    all_names = list(in_names + out_names)
    if partition_name is not None:
        all_names.append(partition_name)
    donate = tuple(range(n_params, n_params + len(out_names)))

    def _body(*args):
        operands = list(args)
        if partition_name is not None:
            operands.append(partition_id_tensor())
        return tuple(
            _bass_exec_p.bind(
                *operands,
                out_avals=tuple(out_avals),
                in_names=tuple(all_names),
                out_names=tuple(out_names),
                lowering_input_output_aliases=(),
                sim_require_finite=True,
                sim_require_nnan=True,
                nc=nc,
            )
        )

    devices = jax.devices()[:N_CORES]
    mesh = Mesh(np.asarray(devices), ("core",))
    spec = NamedSharding(mesh, PartitionSpec("core"))
    n_in = n_params + len(out_names)
    sharded = jax.jit(
        shard_map(
            _body,
            mesh=mesh,
            in_specs=(PartitionSpec("core"),) * n_in,
            out_specs=(PartitionSpec("core"),) * len(out_names),
            check_rep=False,
        ),
        donate_argnums=donate,
        keep_unused=True,
    )
    # in_names order matches dram_tensor declaration order
    staged = [
        jax.device_put(arr, spec)
        for arr in (*full_inputs, *zero_outs)
    ]
    jax.block_until_ready(staged)
    out_arrs = sharded(*staged)
    return np.asarray(out_arrs[0])


def _stage_ab(a: np.ndarray, b: np.ndarray) -> np.ndarray:
    """2x [131072, 512] fp32 -> [256, 128, 4096] fp16 interleaved staging.

    d-major per block: v[core*32 + k, p, h*512 + j] = X[(core*32+k)*512
    + j, h*128 + p] for X in {a (cols 0:2048), b (cols 2048:4096)}. Each
    block is one fully contiguous 1 MiB DRAM region ([a_p | b_p] 8 KiB
    per partition), so input DMA is a single large sequential read per
    block; the d-axis reduction maps to SBUF partitions so the
    TensorEngine can do it with a ones-selector stationary."""
    def dmaj(x):
        v = x.astype(np.float16).reshape(N_CORES * NBLK, BLK, H, P)
        return v.transpose(0, 3, 2, 1).reshape(N_CORES * NBLK, P, CW)
    return np.ascontiguousarray(
        np.concatenate([dmaj(a), dmaj(b)], axis=2)
    )


def kernel(a: np.ndarray, b: np.ndarray) -> np.ndarray:
    nc = _get_nc()
    abf = _stage_ab(
        np.asarray(a, dtype=np.float32).reshape(ROWS_TOTAL, D),
        np.asarray(b, dtype=np.float32).reshape(ROWS_TOTAL, D),
    )
    out = _run_prestaged(nc, abf)
    return out.reshape(B, T).astype(np.float32)
